# revision 1
# baseline (speedup 1.0000x reference)
"""Averaged Hausdorff loss on 8 Trainium2 cores.

Math: d2[i,j] = |x_i|^2 + |y_j|^2 - 2 x_i.y_j computed as a K=5 augmented
matmul on the PE:  lhsT column u_i = [x0,x1,x2,|x|^2,1],
rhs column v_j = [-2y0,-2y1,-2y2,1,|y|^2]  =>  u_i.v_j = d2[i,j].
sqrt is monotonic so mins are taken over d2 and sqrt'd at the end.

Sharding: set1 rows across 8 cores (2048 rows/core vs all 16384 set2 rows).
Each core emits exact row-mins for its shard plus column-min partials;
host combines partials, sqrts, and averages.
"""

import numpy as np
from contextlib import ExitStack

import concourse.bacc as bacc
import concourse.mybir as mybir
import concourse.tile as tile
from concourse import masks
from concourse.bass_utils import run_bass_kernel_spmd

f32 = mybir.dt.float32
N = 16384
M = 16384
NCORES = 8
NLOC = N // NCORES       # 2048 set1 rows per core
RB = NLOC // 128         # 16 row blocks
CHUNK = 512              # matmul free dim (one PSUM bank of f32)
CC = M // CHUNK          # 32 column chunks
TCOLS = M // 128         # 128 transpose tiles for the column-min finish

_compiled = None


def _build():
    nc = bacc.Bacc()
    xa_d = nc.dram_tensor("xa", [5, NLOC], f32, kind="ExternalInput")
    ya_d = nc.dram_tensor("ya", [5, M], f32, kind="ExternalInput")
    rowmin_d = nc.dram_tensor("rowmin", [128, RB], f32, kind="ExternalOutput")
    colmin_d = nc.dram_tensor("colmin", [128, TCOLS], f32, kind="ExternalOutput")

    AX = mybir.AxisListType.X
    MIN = mybir.AluOpType.min

    with tile.TileContext(nc) as tc:
        with ExitStack() as ctx:
            const = ctx.enter_context(tc.tile_pool(name="const", bufs=1))
            iop = ctx.enter_context(tc.tile_pool(name="io", bufs=1))
            rmp = ctx.enter_context(tc.tile_pool(name="rm", bufs=2))
            psmm = ctx.enter_context(tc.tile_pool(name="psmm", bufs=6, space="PSUM"))
            pstp = ctx.enter_context(tc.tile_pool(name="pstp", bufs=2, space="PSUM"))

            ident = const.tile([128, 128], f32)
            masks.make_identity(nc, ident[:])

            xa = iop.tile([5, NLOC], f32)
            nc.sync.dma_start(xa[:], xa_d[:])
            ya = iop.tile([5, M], f32)
            nc.sync.dma_start(ya[:], ya_d[:])

            R = iop.tile([128, M], f32)          # running col-min, d2
            rowmin_sb = iop.tile([128, RB], f32)
            colmin_sb = iop.tile([128, TCOLS], f32)

            for b in range(RB):
                rm = rmp.tile([128, CC], f32, tag="rm")
                for c in range(CC):
                    ps = psmm.tile([128, CHUNK], f32, tag="mm")
                    nc.tensor.matmul(
                        ps[:],
                        xa[:, b * 128:(b + 1) * 128],
                        ya[:, c * CHUNK:(c + 1) * CHUNK],
                        start=True,
                        stop=True,
                    )
                    nc.vector.tensor_reduce(rm[:, c:c + 1], ps[:], axis=AX, op=MIN)
                    Rc = R[:, c * CHUNK:(c + 1) * CHUNK]
                    if b == 0:
                        nc.scalar.copy(Rc, ps[:])
                    else:
                        nc.vector.tensor_tensor(Rc, Rc, ps[:], MIN)
                nc.vector.tensor_reduce(rowmin_sb[:, b:b + 1], rm[:], axis=AX, op=MIN)

            for t in range(TCOLS):
                pt = pstp.tile([128, 128], f32, tag="tp")
                nc.tensor.transpose(pt[:], R[:, t * 128:(t + 1) * 128], ident[:])
                nc.vector.tensor_reduce(colmin_sb[:, t:t + 1], pt[:], axis=AX, op=MIN)

            nc.sync.dma_start(rowmin_d[:], rowmin_sb[:])
            nc.sync.dma_start(colmin_d[:], colmin_sb[:])
    nc.finalize()
    return nc


def _prep_inputs(set1, set2):
    s1 = np.asarray(set1, dtype=np.float32)
    s2 = np.asarray(set2, dtype=np.float32)
    n1 = (s1.astype(np.float64) ** 2).sum(1)[None].astype(np.float32)
    n2 = (s2.astype(np.float64) ** 2).sum(1)[None].astype(np.float32)
    XA = np.concatenate([s1.T, n1, np.ones((1, N), np.float32)], axis=0)
    YR = np.concatenate([-2.0 * s2.T, np.ones((1, M), np.float32), n2], axis=0)
    XA = np.ascontiguousarray(XA, dtype=np.float32)
    YR = np.ascontiguousarray(YR, dtype=np.float32)
    return XA, YR


def _run(nc, XA, YR, trace=False, **kw):
    in_maps = [
        {"xa": np.ascontiguousarray(XA[:, c * NLOC:(c + 1) * NLOC]), "ya": YR}
        for c in range(NCORES)
    ]
    return run_bass_kernel_spmd(nc, in_maps, list(range(NCORES)), trace=trace, **kw)


def _combine(res):
    rowmins, colmins = [], []
    for i in range(NCORES):
        rowmins.append(res.results[i]["rowmin"].T.ravel())
        colmins.append(res.results[i]["colmin"].T.ravel())
    rowmin_d2 = np.concatenate(rowmins)
    colmin_d2 = np.min(np.stack(colmins), axis=0)
    term1 = np.sqrt(np.maximum(rowmin_d2, 0.0)).mean()
    term2 = np.sqrt(np.maximum(colmin_d2, 0.0)).mean()
    return np.asarray(term1 + term2, dtype=np.float32)


def kernel(set1: np.ndarray, set2: np.ndarray) -> np.ndarray:
    global _compiled
    if _compiled is None:
        _compiled = _build()
    XA, YR = _prep_inputs(set1, set2)
    res = _run(_compiled, XA, YR)
    return _combine(res)


# revision 5
# speedup vs baseline: 1.9809x; 1.9809x over previous
"""Averaged Hausdorff loss on 8 Trainium2 cores.

Math: d2[i,j] = |x_i|^2 + |y_j|^2 - 2 x_i.y_j via an augmented inner product
on the PE. fp32 matmul runs at 1/4 rate on TRN2, so each fp32 value is split
into hi+lo fp16 halves (~22 effective mantissa bits) and the product expanded
into K=13 fp16 contraction dims (the xl*yl term, ~1e-6, is dropped):
  dims 0-2 : xh_k * (-2 yh_k)
  dims 3-5 : xh_k * (-2 yl_k)
  dims 6-8 : xl_k * (-2 yh_k)
  dims 9-10: |x|^2 (hi, lo) * 1
  dims 11-12: 1 * |y|^2 (hi, lo)
PSUM accumulates in fp32. sqrt is monotonic so mins are taken over d2 and
sqrt'd on the host.

Sharding: set1 rows across 8 cores (2048 rows/core vs all 16384 set2 rows).
Per (128-row block, 2048-col group): 4 matmuls fill a 4-bank PSUM tile; the
Scalar engine evacuates it to SBUF fp16 (also serving as the col-min init on
the first block); the Vector engine then does
  - row-mins: tensor_tensor_reduce on chunk pairs (elementwise min of two
    512-col chunks + free-axis min-reduce chained through rowmin_sb),
  - col-mins: one fp16 2x tensor_tensor min into the running R16 buffer.
Column partials finish with PE transposes + free-axis reduces; the host
min-combines partials across cores, sqrts, and averages.
"""

import numpy as np
from contextlib import ExitStack

import concourse.bacc as bacc
import concourse.mybir as mybir
import concourse.tile as tile
from concourse import masks
from concourse.bass_utils import run_bass_kernel_spmd

f32 = mybir.dt.float32
f16 = mybir.dt.float16
N = 16384
M = 16384
NCORES = 8
NLOC = N // NCORES       # 2048 set1 rows per core
RB = NLOC // 128         # 16 row blocks
CHUNK = 512              # matmul free dim (one PSUM bank of f32)
GROUP = 4 * CHUNK        # 2048 cols per PSUM tile (4 banks)
NG = M // GROUP          # 8 groups
TCOLS = M // 128         # 128 transpose tiles for the column-min finish
KDIM = 13

_compiled = None


def _build():
    nc = bacc.Bacc()
    xa_d = nc.dram_tensor("xa", [KDIM, NLOC], f16, kind="ExternalInput")
    ya_d = nc.dram_tensor("ya", [KDIM, M], f16, kind="ExternalInput")
    rowmin_d = nc.dram_tensor("rowmin", [128, RB], f32, kind="ExternalOutput")
    colmin_d = nc.dram_tensor("colmin", [128, TCOLS], f32, kind="ExternalOutput")

    AX = mybir.AxisListType.X
    MIN = mybir.AluOpType.min

    with tile.TileContext(nc) as tc:
        with ExitStack() as ctx:
            const = ctx.enter_context(tc.tile_pool(name="const", bufs=1))
            iop = ctx.enter_context(tc.tile_pool(name="io", bufs=1))
            sbp = ctx.enter_context(tc.tile_pool(name="sb16", bufs=3))
            scrp = ctx.enter_context(tc.tile_pool(name="scr", bufs=2))
            psmm = ctx.enter_context(tc.tile_pool(name="psmm", bufs=2, space="PSUM"))

            ident = const.tile([128, 128], f16)
            masks.make_identity(nc, ident[:])

            xa = iop.tile([KDIM, NLOC], f16)
            nc.sync.dma_start(xa[:], xa_d[:])
            ya = iop.tile([KDIM, M], f16)
            nc.sync.dma_start(ya[:], ya_d[:])

            R16 = iop.tile([128, M], f16)        # running col-min, d2, fp16
            rowmin_sb = iop.tile([128, RB], f32)
            rmw = iop.tile([128, RB, NG], f16)
            colmin_sb = iop.tile([128, TCOLS], f32)

            for b in range(RB):
                for g in range(NG):
                    ps = psmm.tile([128, GROUP], f32, tag="mm")
                    for k in range(4):
                        c = g * 4 + k
                        nc.tensor.matmul(
                            ps[:, k * CHUNK:(k + 1) * CHUNK],
                            xa[:, b * 128:(b + 1) * 128],
                            ya[:, c * CHUNK:(c + 1) * CHUNK],
                            start=True,
                            stop=True,
                        )
                    Rg = R16[:, g * GROUP:(g + 1) * GROUP]
                    if b == 0:
                        # evacuate+convert straight into R16 (col-min init)
                        nc.scalar.copy(Rg, ps[:])
                        sb = Rg
                    else:
                        sbt = sbp.tile([128, GROUP], f16, tag="sb16")
                        nc.scalar.copy(sbt[:], ps[:])
                        sb = sbt[:]
                        nc.vector.tensor_tensor(Rg, Rg, sb, MIN)
                    nc.vector.tensor_reduce(
                        rmw[:, b:b + 1, g], sb, axis=AX, op=MIN
                    )

            for b in range(RB):
                nc.vector.tensor_reduce(
                    rowmin_sb[:, b:b + 1], rmw[:, b, :], axis=AX, op=MIN
                )

            for t in range(TCOLS):
                pt = psmm.tile([128, 128], f16, tag="mm")
                nc.tensor.transpose(pt[:], R16[:, t * 128:(t + 1) * 128], ident[:])
                nc.vector.tensor_reduce(colmin_sb[:, t:t + 1], pt[:], axis=AX, op=MIN)

            nc.sync.dma_start(rowmin_d[:], rowmin_sb[:])
            nc.sync.dma_start(colmin_d[:], colmin_sb[:])
    nc.finalize()
    return nc


def _split16(a32):
    """fp32 [k, n] -> (hi, lo) fp16 pair with hi+lo ~ a32 (22-bit mantissa)."""
    hi = a32.astype(np.float16)
    lo = (a32 - hi.astype(np.float32)).astype(np.float16)
    return hi, lo


def _prep_inputs(set1, set2):
    s1 = np.asarray(set1, dtype=np.float32)
    s2 = np.asarray(set2, dtype=np.float32)
    n1 = (s1.astype(np.float64) ** 2).sum(1)[None].astype(np.float32)
    n2 = (s2.astype(np.float64) ** 2).sum(1)[None].astype(np.float32)
    xh, xl = _split16(s1.T)
    yh, yl = _split16(s2.T)
    nxh, nxl = _split16(n1)
    nyh, nyl = _split16(n2)
    m2yh = (-2.0 * yh.astype(np.float32)).astype(np.float16)  # exact
    m2yl = (-2.0 * yl.astype(np.float32)).astype(np.float16)  # exact
    ones_n = np.ones((1, N), np.float16)
    ones_m = np.ones((1, M), np.float16)
    XA = np.concatenate([xh, xh, xl, nxh, nxl, ones_n, ones_n], axis=0)
    YR = np.concatenate([m2yh, m2yl, m2yh, ones_m, ones_m, nyh, nyl], axis=0)
    assert XA.shape == (KDIM, N) and YR.shape == (KDIM, M)
    return np.ascontiguousarray(XA), np.ascontiguousarray(YR)


def _run(nc, XA, YR, trace=False, **kw):
    in_maps = [
        {"xa": np.ascontiguousarray(XA[:, c * NLOC:(c + 1) * NLOC]), "ya": YR}
        for c in range(NCORES)
    ]
    return run_bass_kernel_spmd(nc, in_maps, list(range(NCORES)), trace=trace, **kw)


def _combine(res):
    rowmins, colmins = [], []
    for i in range(NCORES):
        rowmins.append(res.results[i]["rowmin"].T.ravel())
        colmins.append(res.results[i]["colmin"].T.ravel())
    rowmin_d2 = np.concatenate(rowmins).astype(np.float32)
    colmin_d2 = np.min(np.stack(colmins), axis=0).astype(np.float32)
    term1 = np.sqrt(np.maximum(rowmin_d2, 0.0)).mean()
    term2 = np.sqrt(np.maximum(colmin_d2, 0.0)).mean()
    return np.asarray(term1 + term2, dtype=np.float32)


def kernel(set1: np.ndarray, set2: np.ndarray) -> np.ndarray:
    global _compiled
    if _compiled is None:
        _compiled = _build()
    XA, YR = _prep_inputs(set1, set2)
    res = _run(_compiled, XA, YR)
    return _combine(res)


# revision 7
# speedup vs baseline: 2.5020x; 1.2630x over previous
"""Averaged Hausdorff loss on 8 Trainium2 cores.

Math: d2[i,j] = |x_i|^2 + |y_j|^2 - 2 x_i.y_j via an augmented inner product
on the PE. fp32 matmul runs at 1/4 rate on TRN2, so each fp32 value is split
into hi+lo fp16 halves (~22 effective mantissa bits) and the product expanded
into K=13 fp16 contraction dims (the xl*yl term, ~1e-6, is dropped):
  dims 0-2 : xh_k * (-2 yh_k)
  dims 3-5 : xh_k * (-2 yl_k)
  dims 6-8 : xl_k * (-2 yh_k)
  dims 9-10: |x|^2 (hi, lo) * 1
  dims 11-12: 1 * |y|^2 (hi, lo)
PSUM accumulates in fp32. sqrt is monotonic so mins are taken over d2 and
sqrt'd on the host.

Sharding: set1 rows across 8 cores (2048 rows/core vs all 16384 set2 rows).
Per (128-row block, 2048-col group): 4 matmuls fill a 4-bank PSUM tile; the
Scalar engine evacuates it to SBUF fp16 (also serving as the col-min init on
the first block); the Vector engine then does
  - row-mins: tensor_tensor_reduce on chunk pairs (elementwise min of two
    512-col chunks + free-axis min-reduce chained through rowmin_sb),
  - col-mins: one fp16 2x tensor_tensor min into the running R16 buffer.
Column partials finish with PE transposes + free-axis reduces; the host
min-combines partials across cores, sqrts, and averages.
"""

import numpy as np
from contextlib import ExitStack

import concourse.bacc as bacc
import concourse.mybir as mybir
import concourse.tile as tile
from concourse import masks
from concourse.bass_utils import run_bass_kernel_spmd

f32 = mybir.dt.float32
f16 = mybir.dt.float16
N = 16384
M = 16384
NCORES = 8
NLOC = N // NCORES       # 2048 set1 rows per core
RB = NLOC // 128         # 16 row blocks
CHUNK = 512              # matmul free dim (one PSUM bank of f32)
GROUP = 4 * CHUNK        # 2048 cols per PSUM tile (4 banks)
NG = M // GROUP          # 8 groups
TCOLS = M // 128         # 128 transpose tiles for the column-min finish
KDIM = 13

_compiled = None


def _build():
    nc = bacc.Bacc()
    xa_d = nc.dram_tensor("xa", [KDIM, NLOC], f16, kind="ExternalInput")
    ya_d = nc.dram_tensor("ya", [KDIM, M], f16, kind="ExternalInput")
    rowmin_d = nc.dram_tensor("rowmin", [128, RB], f32, kind="ExternalOutput")
    colmin_d = nc.dram_tensor("colmin", [128, TCOLS], f32, kind="ExternalOutput")

    AX = mybir.AxisListType.X
    MIN = mybir.AluOpType.min

    with tile.TileContext(nc) as tc:
        with ExitStack() as ctx:
            const = ctx.enter_context(tc.tile_pool(name="const", bufs=1))
            iop = ctx.enter_context(tc.tile_pool(name="io", bufs=1))
            sbp = ctx.enter_context(tc.tile_pool(name="sb16", bufs=3))
            scrp = ctx.enter_context(tc.tile_pool(name="scr", bufs=2))
            psmm = ctx.enter_context(tc.tile_pool(name="psmm", bufs=2, space="PSUM"))

            ident = const.tile([128, 128], f16)
            masks.make_identity(nc, ident[:])

            xa = iop.tile([KDIM, NLOC], f16)
            nc.sync.dma_start(xa[:], xa_d[:])
            ya = iop.tile([KDIM, M], f16)
            nc.sync.dma_start(ya[:], ya_d[:])

            R16 = iop.tile([128, M], f16)        # running col-min, d2, fp16
            rowmin_sb = iop.tile([128, RB], f32)
            rmw = iop.tile([128, RB, NG], f16)
            colmin_sb = iop.tile([128, TCOLS], f32)

            for b in range(RB):
                for g in range(NG):
                    ps = psmm.tile([128, GROUP], f32, tag="mm")
                    for k in range(4):
                        c = g * 4 + k
                        nc.tensor.matmul(
                            ps[:, k * CHUNK:(k + 1) * CHUNK],
                            xa[:, b * 128:(b + 1) * 128],
                            ya[:, c * CHUNK:(c + 1) * CHUNK],
                            start=True,
                            stop=True,
                        )
                    Rg = R16[:, g * GROUP:(g + 1) * GROUP]
                    if b == 0:
                        # evacuate+convert straight into R16 (col-min init)
                        nc.scalar.copy(Rg, ps[:])
                        sb = Rg
                    else:
                        sbt = sbp.tile([128, GROUP], f16, tag="sb16")
                        nc.scalar.copy(sbt[:], ps[:])
                        sb = sbt[:]
                        nc.vector.tensor_tensor(Rg, Rg, sb, MIN)
                    f1 = scrp.tile([128, 1024], f16, tag="scr")
                    nc.vector.tensor_tensor(f1[:], sb[:, 0:1024], sb[:, 1024:2048], MIN)
                    f2 = scrp.tile([128, 512], f16, tag="scr2")
                    nc.vector.tensor_tensor(f2[:], f1[:, 0:512], f1[:, 512:1024], MIN)
                    nc.vector.tensor_reduce(
                        rmw[:, b:b + 1, g], f2[:], axis=AX, op=MIN
                    )

            for b in range(RB):
                nc.vector.tensor_reduce(
                    rowmin_sb[:, b:b + 1], rmw[:, b, :], axis=AX, op=MIN
                )

            for t4 in range(TCOLS // 4):
                pt = psmm.tile([128, 4, 128], f16, tag="mm")
                for i in range(4):
                    t = 4 * t4 + i
                    nc.tensor.transpose(
                        pt[:, i, :], R16[:, t * 128:(t + 1) * 128], ident[:]
                    )
                nc.vector.tensor_reduce(
                    colmin_sb[:, 4 * t4:4 * t4 + 4], pt[:], axis=AX, op=MIN
                )

            nc.sync.dma_start(rowmin_d[:], rowmin_sb[:])
            nc.sync.dma_start(colmin_d[:], colmin_sb[:])
    nc.finalize()
    return nc


def _split16(a32):
    """fp32 [k, n] -> (hi, lo) fp16 pair with hi+lo ~ a32 (22-bit mantissa)."""
    hi = a32.astype(np.float16)
    lo = (a32 - hi.astype(np.float32)).astype(np.float16)
    return hi, lo


def _prep_inputs(set1, set2):
    s1 = np.asarray(set1, dtype=np.float32)
    s2 = np.asarray(set2, dtype=np.float32)
    n1 = (s1.astype(np.float64) ** 2).sum(1)[None].astype(np.float32)
    n2 = (s2.astype(np.float64) ** 2).sum(1)[None].astype(np.float32)
    xh, xl = _split16(s1.T)
    yh, yl = _split16(s2.T)
    nxh, nxl = _split16(n1)
    nyh, nyl = _split16(n2)
    m2yh = (-2.0 * yh.astype(np.float32)).astype(np.float16)  # exact
    m2yl = (-2.0 * yl.astype(np.float32)).astype(np.float16)  # exact
    ones_n = np.ones((1, N), np.float16)
    ones_m = np.ones((1, M), np.float16)
    XA = np.concatenate([xh, xh, xl, nxh, nxl, ones_n, ones_n], axis=0)
    YR = np.concatenate([m2yh, m2yl, m2yh, ones_m, ones_m, nyh, nyl], axis=0)
    assert XA.shape == (KDIM, N) and YR.shape == (KDIM, M)
    return np.ascontiguousarray(XA), np.ascontiguousarray(YR)


def _run(nc, XA, YR, trace=False, **kw):
    in_maps = [
        {"xa": np.ascontiguousarray(XA[:, c * NLOC:(c + 1) * NLOC]), "ya": YR}
        for c in range(NCORES)
    ]
    return run_bass_kernel_spmd(nc, in_maps, list(range(NCORES)), trace=trace, **kw)


def _combine(res):
    rowmins, colmins = [], []
    for i in range(NCORES):
        rowmins.append(res.results[i]["rowmin"].T.ravel())
        colmins.append(res.results[i]["colmin"].T.ravel())
    rowmin_d2 = np.concatenate(rowmins).astype(np.float32)
    colmin_d2 = np.min(np.stack(colmins), axis=0).astype(np.float32)
    term1 = np.sqrt(np.maximum(rowmin_d2, 0.0)).mean()
    term2 = np.sqrt(np.maximum(colmin_d2, 0.0)).mean()
    return np.asarray(term1 + term2, dtype=np.float32)


def kernel(set1: np.ndarray, set2: np.ndarray) -> np.ndarray:
    global _compiled
    if _compiled is None:
        _compiled = _build()
    XA, YR = _prep_inputs(set1, set2)
    res = _run(_compiled, XA, YR)
    return _combine(res)


# revision 9
# speedup vs baseline: 2.5026x; 1.0003x over previous
"""Averaged Hausdorff loss on 8 Trainium2 cores.

Math: d2[i,j] = |x_i|^2 + |y_j|^2 - 2 x_i.y_j via an augmented inner product
on the PE. fp32 matmul runs at 1/4 rate on TRN2, so each fp32 value is split
into hi+lo fp16 halves (~22 effective mantissa bits) and the product expanded
into K=13 fp16 contraction dims (the xl*yl term, ~1e-6, is dropped):
  dims 0-2 : xh_k * (-2 yh_k)
  dims 3-5 : xh_k * (-2 yl_k)
  dims 6-8 : xl_k * (-2 yh_k)
  dims 9-10: |x|^2 (hi, lo) * 1
  dims 11-12: 1 * |y|^2 (hi, lo)
PSUM accumulates in fp32. sqrt is monotonic so mins are taken over d2 and
sqrt'd on the host.

Sharding: set1 rows across 8 cores (2048 rows/core vs all 16384 set2 rows).
Per (128-row block, 2048-col group): 4 matmuls fill a 4-bank PSUM tile; the
Scalar engine evacuates it to SBUF fp16 (also serving as the col-min init on
the first block); the Vector engine then does
  - row-mins: tensor_tensor_reduce on chunk pairs (elementwise min of two
    512-col chunks + free-axis min-reduce chained through rowmin_sb),
  - col-mins: one fp16 2x tensor_tensor min into the running R16 buffer.
Column partials finish with PE transposes + free-axis reduces; the host
min-combines partials across cores, sqrts, and averages.
"""

import numpy as np
from contextlib import ExitStack

import concourse.bacc as bacc
import concourse.mybir as mybir
import concourse.tile as tile
from concourse import masks
from concourse.bass_utils import run_bass_kernel_spmd

f32 = mybir.dt.float32
f16 = mybir.dt.float16
N = 16384
M = 16384
NCORES = 8
NLOC = N // NCORES       # 2048 set1 rows per core
RB = NLOC // 128         # 16 row blocks
CHUNK = 512              # matmul free dim (one PSUM bank of f32)
GROUP = 4 * CHUNK        # 2048 cols per PSUM tile (4 banks)
NG = M // GROUP          # 8 groups
TCOLS = M // 128         # 128 transpose tiles for the column-min finish
KDIM = 13

_compiled = None


def _build():
    nc = bacc.Bacc()
    xa_d = nc.dram_tensor("xa", [KDIM, NLOC], f16, kind="ExternalInput")
    ya_d = nc.dram_tensor("ya", [KDIM, M], f16, kind="ExternalInput")
    rowmin_d = nc.dram_tensor("rowmin", [128, RB], f32, kind="ExternalOutput")
    colmin_d = nc.dram_tensor("colmin", [128, TCOLS], f32, kind="ExternalOutput")

    AX = mybir.AxisListType.X
    MIN = mybir.AluOpType.min

    with tile.TileContext(nc) as tc:
        with ExitStack() as ctx:
            const = ctx.enter_context(tc.tile_pool(name="const", bufs=1))
            iop = ctx.enter_context(tc.tile_pool(name="io", bufs=1))
            sbp = ctx.enter_context(tc.tile_pool(name="sb16", bufs=3))
            scrp = ctx.enter_context(tc.tile_pool(name="scr", bufs=2))
            psmm = ctx.enter_context(tc.tile_pool(name="psmm", bufs=2, space="PSUM"))

            ident = const.tile([128, 128], f16)
            masks.make_identity(nc, ident[:])

            xa = iop.tile([KDIM, NLOC], f16)
            nc.sync.dma_start(xa[:], xa_d[:])
            ya = iop.tile([KDIM, M], f16)
            nc.sync.dma_start(ya[:], ya_d[:])

            R16 = iop.tile([128, M], f16)        # running col-min, d2, fp16
            rowmin_sb = iop.tile([128, RB], f32)
            rmw = iop.tile([128, RB, NG], f16)
            colmin_sb = iop.tile([128, TCOLS], f32)

            for b in range(RB):
                for g in range(NG):
                    ps = psmm.tile([128, GROUP], f32, tag="mm")
                    for k in range(4):
                        c = g * 4 + k
                        nc.tensor.matmul(
                            ps[:, k * CHUNK:(k + 1) * CHUNK],
                            xa[:, b * 128:(b + 1) * 128],
                            ya[:, c * CHUNK:(c + 1) * CHUNK],
                            start=True,
                            stop=True,
                        )
                    Rg = R16[:, g * GROUP:(g + 1) * GROUP]
                    if b == 0:
                        # evacuate+convert straight into R16 (col-min init)
                        nc.scalar.copy(Rg, ps[:])
                        sb = Rg
                    else:
                        sbt = sbp.tile([128, GROUP], f16, tag="sb16")
                        nc.scalar.copy(sbt[:], ps[:])
                        sb = sbt[:]
                        nc.vector.tensor_tensor(Rg, Rg, sb, MIN)
                    f1 = scrp.tile([128, 1024], f16, tag="scr")
                    nc.vector.tensor_tensor(f1[:], sb[:, 0:1024], sb[:, 1024:2048], MIN)
                    f2 = scrp.tile([128, 512], f16, tag="scr2")
                    nc.vector.tensor_tensor(f2[:], f1[:, 0:512], f1[:, 512:1024], MIN)
                    nc.vector.tensor_reduce(
                        rmw[:, b:b + 1, g], f2[:], axis=AX, op=MIN
                    )

            for b in range(RB):
                nc.vector.tensor_reduce(
                    rowmin_sb[:, b:b + 1], rmw[:, b, :], axis=AX, op=MIN
                )

            for t4 in range(TCOLS // 4):
                pt = psmm.tile([128, 4, 128], f16, tag="mm")
                for i in range(4):
                    t = 4 * t4 + i
                    nc.tensor.transpose(
                        pt[:, i, :], R16[:, t * 128:(t + 1) * 128], ident[:]
                    )
                nc.vector.tensor_reduce(
                    colmin_sb[:, 4 * t4:4 * t4 + 4], pt[:], axis=AX, op=MIN
                )

            nc.sync.dma_start(rowmin_d[:], rowmin_sb[:])
            nc.sync.dma_start(colmin_d[:], colmin_sb[:])
    nc.finalize()
    return nc


def _split16(a32):
    """fp32 [k, n] -> (hi, lo) fp16 pair with hi+lo ~ a32 (22-bit mantissa)."""
    hi = a32.astype(np.float16)
    lo = (a32 - hi.astype(np.float32)).astype(np.float16)
    return hi, lo


def _prep_inputs(set1, set2):
    s1 = np.asarray(set1, dtype=np.float32)
    s2 = np.asarray(set2, dtype=np.float32)
    n1 = (s1.astype(np.float64) ** 2).sum(1)[None].astype(np.float32)
    n2 = (s2.astype(np.float64) ** 2).sum(1)[None].astype(np.float32)
    xh, xl = _split16(s1.T)
    yh, yl = _split16(s2.T)
    nxh, nxl = _split16(n1)
    nyh, nyl = _split16(n2)
    m2yh = (-2.0 * yh.astype(np.float32)).astype(np.float16)  # exact
    m2yl = (-2.0 * yl.astype(np.float32)).astype(np.float16)  # exact
    ones_n = np.ones((1, N), np.float16)
    ones_m = np.ones((1, M), np.float16)
    XA = np.concatenate([xh, xh, xl, nxh, nxl, ones_n, ones_n], axis=0)
    YR = np.concatenate([m2yh, m2yl, m2yh, ones_m, ones_m, nyh, nyl], axis=0)
    assert XA.shape == (KDIM, N) and YR.shape == (KDIM, M)
    return np.ascontiguousarray(XA), np.ascontiguousarray(YR)


def _run(nc, XA, YR, trace=False, **kw):
    in_maps = [
        {"xa": np.ascontiguousarray(XA[:, c * NLOC:(c + 1) * NLOC]), "ya": YR}
        for c in range(NCORES)
    ]
    return run_bass_kernel_spmd(nc, in_maps, list(range(NCORES)), trace=trace, **kw)


def _combine(res):
    rowmins, colmins = [], []
    for i in range(NCORES):
        rowmins.append(res.results[i]["rowmin"].T.ravel())
        colmins.append(res.results[i]["colmin"].T.ravel())
    rowmin_d2 = np.concatenate(rowmins).astype(np.float32)
    colmin_d2 = np.min(np.stack(colmins), axis=0).astype(np.float32)
    term1 = np.sqrt(np.maximum(rowmin_d2, 0.0)).mean()
    term2 = np.sqrt(np.maximum(colmin_d2, 0.0)).mean()
    return np.asarray(term1 + term2, dtype=np.float32)


def kernel(set1: np.ndarray, set2: np.ndarray) -> np.ndarray:
    global _compiled
    if _compiled is None:
        _compiled = _build()
    XA, YR = _prep_inputs(set1, set2)
    res = _run(_compiled, XA, YR)
    return _combine(res)
